# revision 22
# baseline (speedup 1.0000x reference)
"""Trainium2 Bass kernel for nn_HammingL2 (pairwise Hamming-weighted L2 loss).

Math: per-LUT loss = sum_{i<j} W[i,j](v_i-v_j)^2 = d.(v*v) - v^T W v with
d = rowsum(W).  Summed over all LUTs this equals  sum_ij M_ij G_ij  where
G = V^T V  (Gram over all LUTs, [256,256]) and  M = diag(d) - W.

Strategy: data-parallel over 8 NeuronCores.  Each core streams its
[8192, 256] shard of `luts` from HBM and accumulates the shard Gram
G_c = V_c^T V_c on the tensor engine (128 accumulating matmuls into two
[128,256] PSUM tiles, operands bitcast to float32r for single-pass fp32
matmul speed).  The raw Gram is copied to SBUF and DMA'd out; the host
computes sum(M * sum_c G_c) / NUM_LUTS (a 256x256 reduction - trivial).

The kernel is DMA-bound: 8 MiB/core of f32 reads at ~300-380 GB/s/core.
"""

import numpy as np

N_CORES = 8
NUM_LUTS = 65536
L = 256               # LUT_SIZE
SHARD = NUM_LUTS // N_CORES   # 8192 LUTs per core
P = 128               # partitions
CHUNKS = SHARD // P   # 64 matmul chunks per core

# DMA block sizes in chunks (1 chunk = 128 LUT rows = [128, 256] f32 = 128 KiB).
# Within a block of q chunks, partition p holds q CONSECUTIVE shard rows
# (r0 + p*q + c) so each partition's DMA run is q KiB contiguous -> much
# better descriptor efficiency than 1 KiB runs.  Tapered tail so the PE
# drains right behind the last byte.
BLOCK_SIZES = [8] * 7 + [4, 2, 1, 1]
assert sum(BLOCK_SIZES) == CHUNKS

MODE = "f32r"         # matmul operand mode: "f32" | "f32r" | "bf16"

_CACHE = {}


def _seed_ntff_hook():
    """Make `antenv.axon_hooks` importable so run_bass_kernel_spmd(trace=True)
    can capture NTFF profiles under axon.  No-op if already present."""
    import sys
    import types

    try:
        import antenv.axon_hooks  # noqa: F401
        return
    except Exception:
        pass
    mod = types.ModuleType("antenv.axon_hooks")
    mod._hook = None

    def set_axon_ntff_profile_hook(h):
        mod._hook = h

    def get_axon_ntff_profile_hook():
        if mod._hook is None:
            try:
                from trn_agent_boot.trn_boot import _ntff_profile_via_ctypes

                mod._hook = _ntff_profile_via_ctypes("/opt/axon/libaxon_pjrt.so")
            except Exception:
                return None
        return mod._hook

    mod.set_axon_ntff_profile_hook = set_axon_ntff_profile_hook
    mod.get_axon_ntff_profile_hook = get_axon_ntff_profile_hook
    sys.modules["antenv.axon_hooks"] = mod


def _build(mode=None):
    """Build + compile the per-core Bass kernel (cached)."""
    mode = mode or MODE
    if mode in _CACHE:
        return _CACHE[mode]

    import concourse.mybir as mybir
    import concourse.tile as tile
    from concourse import bacc

    f32 = mybir.dt.float32
    bf16 = mybir.dt.bfloat16
    v_dt = mybir.dt.float32r if mode == "f32r" else f32
    nc = bacc.Bacc("TRN2", target_bir_lowering=False, debug=False, num_devices=N_CORES)
    v = nc.dram_tensor("v", [SHARD, L], v_dt, kind="ExternalInput").ap()
    out = nc.dram_tensor("out", [P, 2, L], f32, kind="ExternalOutput").ap()

    max_q = max(BLOCK_SIZES)

    with tile.TileContext(nc) as tc:
        with (
            tc.tile_pool(name="vpool", bufs=len(BLOCK_SIZES)) as vpool,
            tc.tile_pool(name="psum", bufs=2, space="PSUM") as psum_pool,
            tc.tile_pool(name="opool", bufs=1) as opool,
        ):
            g_ps = [
                psum_pool.tile([P, L], f32, tag=f"g{h}", name=f"g{h}")
                for h in range(2)
            ]

            k = 0
            r0 = 0
            for blk in BLOCK_SIZES:
                # partition p <- rows r0 + p*blk + [0, blk): blk KiB contiguous
                src = v[r0 : r0 + P * blk].rearrange("(p q) j -> p q j", q=blk)
                if mode == "bf16":
                    vt = vpool.tile([P, max_q, L], bf16, tag="v", name="vt")
                    nc.gpsimd.dma_start(vt[:, :blk, :], src)
                else:
                    vt = vpool.tile([P, max_q, L], v_dt, tag="v", name="vt")
                    nc.sync.dma_start(vt[:, :blk, :], src)
                for c in range(blk):
                    rhs = vt[:, c, :]
                    for h in range(2):
                        nc.tensor.matmul(
                            g_ps[h][:],
                            vt[:, c, h * P : (h + 1) * P],
                            rhs,
                            start=(k == 0),
                            stop=(k == CHUNKS - 1),
                        )
                    k += 1
                r0 += P * blk

            o_tile = opool.tile([P, 2, L], f32, tag="o")
            for h in range(2):
                nc.vector.tensor_copy(o_tile[:, h, :], g_ps[h][:])
            nc.sync.dma_start(out, o_tile[:])

    nc.compile()
    _CACHE[mode] = nc
    return nc


def _run(luts, W, trace=False, mode=None, **trace_kwargs):
    """Shard, run on 8 cores, return (loss_scalar, BassKernelResults)."""
    _seed_ntff_hook()
    from concourse.bass_utils import run_bass_kernel_spmd

    nc = _build(mode)

    luts = np.ascontiguousarray(np.asarray(luts, dtype=np.float32))
    W = np.asarray(W, dtype=np.float32)

    in_maps = [{"v": luts[i * SHARD : (i + 1) * SHARD]} for i in range(N_CORES)]
    res = run_bass_kernel_spmd(
        nc, in_maps, core_ids=list(range(N_CORES)), trace=trace, **trace_kwargs
    )

    # host epilogue: loss = sum(M * G_total) / NUM_LUTS  (256x256 - trivial)
    Wd = W.astype(np.float64)
    M = np.diag(Wd.sum(axis=1)) - Wd
    G = np.zeros((L, L), dtype=np.float64)
    for r in res.results:
        g = r["out"].astype(np.float64)  # [128, 2, 256]
        G[:P] += g[:, 0, :]
        G[P:] += g[:, 1, :]
    loss = np.asarray((M * G).sum() / NUM_LUTS, dtype=np.float32)
    return loss, res


def kernel(luts, W, gamma=None, **_unused):
    loss, _ = _run(luts, W, trace=False)
    return loss


if __name__ == "__main__":
    rng = np.random.default_rng(0)
    luts = rng.standard_normal((NUM_LUTS, L), dtype=np.float32)
    W = rng.random((L, L), dtype=np.float32)
    W = (W + W.T) / 2
    np.fill_diagonal(W, 0.0)
    print(kernel(luts, W))


# revision 23
# speedup vs baseline: 1.0431x; 1.0431x over previous
"""Trainium2 Bass kernel for nn_HammingL2 (pairwise Hamming-weighted L2 loss).

Math: per-LUT loss = sum_{i<j} W[i,j](v_i-v_j)^2 = d.(v*v) - v^T W v with
d = rowsum(W).  Summed over all LUTs this equals  sum_ij M_ij G_ij  where
G = V^T V  (Gram over all LUTs, [256,256]) and  M = diag(d) - W.

Strategy: data-parallel over 8 NeuronCores.  Each core streams its
[8192, 256] shard of `luts` from HBM and accumulates the shard Gram
G_c = V_c^T V_c on the tensor engine (128 accumulating matmuls into two
[128,256] PSUM tiles, operands bitcast to float32r for single-pass fp32
matmul speed).  The raw Gram is copied to SBUF and DMA'd out; the host
computes sum(M * sum_c G_c) / NUM_LUTS (a 256x256 reduction - trivial).

The kernel is DMA-bound: 8 MiB/core of f32 reads at ~300-380 GB/s/core.
"""

import numpy as np

N_CORES = 8
NUM_LUTS = 65536
L = 256               # LUT_SIZE
SHARD = NUM_LUTS // N_CORES   # 8192 LUTs per core
P = 128               # partitions
CHUNKS = SHARD // P   # 64 matmul chunks per core

# DMA block sizes in chunks (1 chunk = 128 LUT rows = [128, 256] f32 = 128 KiB).
# Within a block of q chunks, partition p holds q CONSECUTIVE shard rows
# (r0 + p*q + c) so each partition's DMA run is q KiB contiguous -> much
# better descriptor efficiency than 1 KiB runs.  Tapered tail so the PE
# drains right behind the last byte.
BLOCK_SIZES = [4] * 15 + [2, 1, 1]
assert sum(BLOCK_SIZES) == CHUNKS

MODE = "f32r"         # matmul operand mode: "f32" | "f32r" | "bf16"

_CACHE = {}


def _seed_ntff_hook():
    """Make `antenv.axon_hooks` importable so run_bass_kernel_spmd(trace=True)
    can capture NTFF profiles under axon.  No-op if already present."""
    import sys
    import types

    try:
        import antenv.axon_hooks  # noqa: F401
        return
    except Exception:
        pass
    mod = types.ModuleType("antenv.axon_hooks")
    mod._hook = None

    def set_axon_ntff_profile_hook(h):
        mod._hook = h

    def get_axon_ntff_profile_hook():
        if mod._hook is None:
            try:
                from trn_agent_boot.trn_boot import _ntff_profile_via_ctypes

                mod._hook = _ntff_profile_via_ctypes("/opt/axon/libaxon_pjrt.so")
            except Exception:
                return None
        return mod._hook

    mod.set_axon_ntff_profile_hook = set_axon_ntff_profile_hook
    mod.get_axon_ntff_profile_hook = get_axon_ntff_profile_hook
    sys.modules["antenv.axon_hooks"] = mod


def _build(mode=None):
    """Build + compile the per-core Bass kernel (cached)."""
    mode = mode or MODE
    if mode in _CACHE:
        return _CACHE[mode]

    import concourse.mybir as mybir
    import concourse.tile as tile
    from concourse import bacc

    f32 = mybir.dt.float32
    bf16 = mybir.dt.bfloat16
    v_dt = mybir.dt.float32r if mode == "f32r" else f32
    nc = bacc.Bacc("TRN2", target_bir_lowering=False, debug=False, num_devices=N_CORES)
    v = nc.dram_tensor("v", [SHARD, L], v_dt, kind="ExternalInput").ap()
    out = nc.dram_tensor("out", [P, 2, L], f32, kind="ExternalOutput").ap()

    max_q = max(BLOCK_SIZES)

    with tile.TileContext(nc) as tc:
        with (
            tc.tile_pool(name="vpool", bufs=len(BLOCK_SIZES)) as vpool,
            tc.tile_pool(name="psum", bufs=2, space="PSUM") as psum_pool,
            tc.tile_pool(name="opool", bufs=1) as opool,
        ):
            g_ps = [
                psum_pool.tile([P, L], f32, tag=f"g{h}", name=f"g{h}")
                for h in range(2)
            ]

            k = 0
            r0 = 0
            for blk in BLOCK_SIZES:
                # partition p <- rows r0 + p*blk + [0, blk): blk KiB contiguous
                src = v[r0 : r0 + P * blk].rearrange("(p q) j -> p q j", q=blk)
                if mode == "bf16":
                    vt = vpool.tile([P, max_q, L], bf16, tag="v", name="vt")
                    nc.gpsimd.dma_start(vt[:, :blk, :], src)
                else:
                    vt = vpool.tile([P, max_q, L], v_dt, tag="v", name="vt")
                    nc.sync.dma_start(vt[:, :blk, :], src)
                for c in range(blk):
                    rhs = vt[:, c, :]
                    for h in range(2):
                        nc.tensor.matmul(
                            g_ps[h][:],
                            vt[:, c, h * P : (h + 1) * P],
                            rhs,
                            start=(k == 0),
                            stop=(k == CHUNKS - 1),
                        )
                    k += 1
                r0 += P * blk

            o_tile = opool.tile([P, 2, L], f32, tag="o")
            for h in range(2):
                nc.vector.tensor_copy(o_tile[:, h, :], g_ps[h][:])
            nc.sync.dma_start(out, o_tile[:])

    nc.compile()
    _CACHE[mode] = nc
    return nc


def _run(luts, W, trace=False, mode=None, **trace_kwargs):
    """Shard, run on 8 cores, return (loss_scalar, BassKernelResults)."""
    _seed_ntff_hook()
    from concourse.bass_utils import run_bass_kernel_spmd

    nc = _build(mode)

    luts = np.ascontiguousarray(np.asarray(luts, dtype=np.float32))
    W = np.asarray(W, dtype=np.float32)

    in_maps = [{"v": luts[i * SHARD : (i + 1) * SHARD]} for i in range(N_CORES)]
    res = run_bass_kernel_spmd(
        nc, in_maps, core_ids=list(range(N_CORES)), trace=trace, **trace_kwargs
    )

    # host epilogue: loss = sum(M * G_total) / NUM_LUTS  (256x256 - trivial)
    Wd = W.astype(np.float64)
    M = np.diag(Wd.sum(axis=1)) - Wd
    G = np.zeros((L, L), dtype=np.float64)
    for r in res.results:
        g = r["out"].astype(np.float64)  # [128, 2, 256]
        G[:P] += g[:, 0, :]
        G[P:] += g[:, 1, :]
    loss = np.asarray((M * G).sum() / NUM_LUTS, dtype=np.float32)
    return loss, res


def kernel(luts, W, gamma=None, **_unused):
    loss, _ = _run(luts, W, trace=False)
    return loss


if __name__ == "__main__":
    rng = np.random.default_rng(0)
    luts = rng.standard_normal((NUM_LUTS, L), dtype=np.float32)
    W = rng.random((L, L), dtype=np.float32)
    W = (W + W.T) / 2
    np.fill_diagonal(W, 0.0)
    print(kernel(luts, W))


# revision 24
# speedup vs baseline: 1.0823x; 1.0376x over previous
"""Trainium2 Bass kernel for nn_HammingL2 (pairwise Hamming-weighted L2 loss).

Math: per-LUT loss = sum_{i<j} W[i,j](v_i-v_j)^2 = d.(v*v) - v^T W v with
d = rowsum(W).  Summed over all LUTs this equals  sum_ij M_ij G_ij  where
G = V^T V  (Gram over all LUTs, [256,256]) and  M = diag(d) - W.

Strategy: data-parallel over 8 NeuronCores.  Each core streams its
[8192, 256] shard of `luts` from HBM and accumulates the shard Gram
G_c = V_c^T V_c on the tensor engine (128 accumulating matmuls into two
[128,256] PSUM tiles, operands bitcast to float32r for single-pass fp32
matmul speed).  The raw Gram is copied to SBUF and DMA'd out; the host
computes sum(M * sum_c G_c) / NUM_LUTS (a 256x256 reduction - trivial).

The kernel is DMA-bound: 8 MiB/core of f32 reads at ~300-380 GB/s/core.
"""

import numpy as np

N_CORES = 8
NUM_LUTS = 65536
L = 256               # LUT_SIZE
SHARD = NUM_LUTS // N_CORES   # 8192 LUTs per core
P = 128               # partitions
CHUNKS = SHARD // P   # 64 matmul chunks per core

# DMA block sizes in chunks (1 chunk = 128 LUT rows = [128, 256] f32 = 128 KiB).
# Within a block of q chunks, partition p holds q CONSECUTIVE shard rows
# (r0 + p*q + c) so each partition's DMA run is q KiB contiguous -> much
# better descriptor efficiency than 1 KiB runs.  Tapered tail so the PE
# drains right behind the last byte.
BLOCK_SIZES = [4] * 15 + [2, 1, 1]
assert sum(BLOCK_SIZES) == CHUNKS

MODE = "f32r"         # matmul operand mode: "f32" | "f32r" | "bf16"

_CACHE = {}


def _seed_ntff_hook():
    """Make `antenv.axon_hooks` importable so run_bass_kernel_spmd(trace=True)
    can capture NTFF profiles under axon.  No-op if already present."""
    import sys
    import types

    try:
        import antenv.axon_hooks  # noqa: F401
        return
    except Exception:
        pass
    mod = types.ModuleType("antenv.axon_hooks")
    mod._hook = None

    def set_axon_ntff_profile_hook(h):
        mod._hook = h

    def get_axon_ntff_profile_hook():
        if mod._hook is None:
            try:
                from trn_agent_boot.trn_boot import _ntff_profile_via_ctypes

                mod._hook = _ntff_profile_via_ctypes("/opt/axon/libaxon_pjrt.so")
            except Exception:
                return None
        return mod._hook

    mod.set_axon_ntff_profile_hook = set_axon_ntff_profile_hook
    mod.get_axon_ntff_profile_hook = get_axon_ntff_profile_hook
    sys.modules["antenv.axon_hooks"] = mod


def _build(mode=None):
    """Build + compile the per-core Bass kernel (cached)."""
    mode = mode or MODE
    if mode in _CACHE:
        return _CACHE[mode]

    import concourse.mybir as mybir
    import concourse.tile as tile
    from concourse import bacc

    f32 = mybir.dt.float32
    bf16 = mybir.dt.bfloat16
    v_dt = mybir.dt.float32r if mode == "f32r" else f32
    nc = bacc.Bacc("TRN2", target_bir_lowering=False, debug=False, num_devices=N_CORES)
    v = nc.dram_tensor("v", [SHARD, L], v_dt, kind="ExternalInput").ap()
    out = nc.dram_tensor("out", [P, 2, L], f32, kind="ExternalOutput").ap()

    max_q = max(BLOCK_SIZES)

    with tile.TileContext(nc) as tc:
        with (
            tc.tile_pool(name="vpool", bufs=len(BLOCK_SIZES)) as vpool,
            tc.tile_pool(name="psum", bufs=2, space="PSUM") as psum_pool,
            tc.tile_pool(name="opool", bufs=1) as opool,
        ):
            g_ps = [
                psum_pool.tile([P, L], f32, tag=f"g{h}", name=f"g{h}")
                for h in range(2)
            ]

            k = 0
            r0 = 0
            for bi, blk in enumerate(BLOCK_SIZES):
                # partition p <- rows r0 + p*blk + [0, blk): blk KiB contiguous
                src = v[r0 : r0 + P * blk].rearrange("(p q) j -> p q j", q=blk)
                if mode == "bf16":
                    vt = vpool.tile([P, max_q, L], bf16, tag="v", name="vt")
                    nc.gpsimd.dma_start(vt[:, :blk, :], src)
                else:
                    vt = vpool.tile([P, max_q, L], v_dt, tag="v", name="vt")
                    # alternate between the two HWDGE rings (SP / ACT)
                    eng = nc.sync if bi % 2 == 0 else nc.scalar
                    eng.dma_start(vt[:, :blk, :], src)
                for c in range(blk):
                    rhs = vt[:, c, :]
                    for h in range(2):
                        nc.tensor.matmul(
                            g_ps[h][:],
                            vt[:, c, h * P : (h + 1) * P],
                            rhs,
                            start=(k == 0),
                            stop=(k == CHUNKS - 1),
                        )
                    k += 1
                r0 += P * blk

            o_tile = opool.tile([P, 2, L], f32, tag="o")
            for h in range(2):
                nc.vector.tensor_copy(o_tile[:, h, :], g_ps[h][:])
            nc.sync.dma_start(out, o_tile[:])

    nc.compile()
    _CACHE[mode] = nc
    return nc


def _run(luts, W, trace=False, mode=None, **trace_kwargs):
    """Shard, run on 8 cores, return (loss_scalar, BassKernelResults)."""
    _seed_ntff_hook()
    from concourse.bass_utils import run_bass_kernel_spmd

    nc = _build(mode)

    luts = np.ascontiguousarray(np.asarray(luts, dtype=np.float32))
    W = np.asarray(W, dtype=np.float32)

    in_maps = [{"v": luts[i * SHARD : (i + 1) * SHARD]} for i in range(N_CORES)]
    res = run_bass_kernel_spmd(
        nc, in_maps, core_ids=list(range(N_CORES)), trace=trace, **trace_kwargs
    )

    # host epilogue: loss = sum(M * G_total) / NUM_LUTS  (256x256 - trivial)
    Wd = W.astype(np.float64)
    M = np.diag(Wd.sum(axis=1)) - Wd
    G = np.zeros((L, L), dtype=np.float64)
    for r in res.results:
        g = r["out"].astype(np.float64)  # [128, 2, 256]
        G[:P] += g[:, 0, :]
        G[P:] += g[:, 1, :]
    loss = np.asarray((M * G).sum() / NUM_LUTS, dtype=np.float32)
    return loss, res


def kernel(luts, W, gamma=None, **_unused):
    loss, _ = _run(luts, W, trace=False)
    return loss


if __name__ == "__main__":
    rng = np.random.default_rng(0)
    luts = rng.standard_normal((NUM_LUTS, L), dtype=np.float32)
    W = rng.random((L, L), dtype=np.float32)
    W = (W + W.T) / 2
    np.fill_diagonal(W, 0.0)
    print(kernel(luts, W))


# revision 26
# speedup vs baseline: 1.0924x; 1.0094x over previous
"""Trainium2 Bass kernel for nn_HammingL2 (pairwise Hamming-weighted L2 loss).

Math: per-LUT loss = sum_{i<j} W[i,j](v_i-v_j)^2 = d.(v*v) - v^T W v with
d = rowsum(W).  Summed over all LUTs this equals  sum_ij M_ij G_ij  where
G = V^T V  (Gram over all LUTs, [256,256]) and  M = diag(d) - W.

Strategy: data-parallel over 8 NeuronCores.  Each core streams its
[8192, 256] shard of `luts` from HBM and accumulates the shard Gram
G_c = V_c^T V_c on the tensor engine (128 accumulating matmuls into two
[128,256] PSUM tiles, operands bitcast to float32r for single-pass fp32
matmul speed).  The raw Gram is copied to SBUF and DMA'd out; the host
computes sum(M * sum_c G_c) / NUM_LUTS (a 256x256 reduction - trivial).

The kernel is DMA-bound: 8 MiB/core of f32 reads at ~300-380 GB/s/core.
"""

import numpy as np

N_CORES = 8
NUM_LUTS = 65536
L = 256               # LUT_SIZE
SHARD = NUM_LUTS // N_CORES   # 8192 LUTs per core
P = 128               # partitions
CHUNKS = SHARD // P   # 64 matmul chunks per core

# DMA block sizes in chunks (1 chunk = 128 LUT rows = [128, 256] f32 = 128 KiB).
# Within a block of q chunks, partition p holds q CONSECUTIVE shard rows
# (r0 + p*q + c) so each partition's DMA run is q KiB contiguous -> much
# better descriptor efficiency than 1 KiB runs.  Tapered tail so the PE
# drains right behind the last byte.
BLOCK_SIZES = [4] * 15 + [2, 1, 1]
assert sum(BLOCK_SIZES) == CHUNKS

MODE = "f32r"         # matmul operand mode: "f32" | "f32r" | "bf16"

_CACHE = {}


def _seed_ntff_hook():
    """Make `antenv.axon_hooks` importable so run_bass_kernel_spmd(trace=True)
    can capture NTFF profiles under axon.  No-op if already present."""
    import sys
    import types

    try:
        import antenv.axon_hooks  # noqa: F401
        return
    except Exception:
        pass
    mod = types.ModuleType("antenv.axon_hooks")
    mod._hook = None

    def set_axon_ntff_profile_hook(h):
        mod._hook = h

    def get_axon_ntff_profile_hook():
        if mod._hook is None:
            try:
                from trn_agent_boot.trn_boot import _ntff_profile_via_ctypes

                mod._hook = _ntff_profile_via_ctypes("/opt/axon/libaxon_pjrt.so")
            except Exception:
                return None
        return mod._hook

    mod.set_axon_ntff_profile_hook = set_axon_ntff_profile_hook
    mod.get_axon_ntff_profile_hook = get_axon_ntff_profile_hook
    sys.modules["antenv.axon_hooks"] = mod


def _build(mode=None):
    """Build + compile the per-core Bass kernel (cached)."""
    mode = mode or MODE
    if mode in _CACHE:
        return _CACHE[mode]

    import concourse.mybir as mybir
    import concourse.tile as tile
    from concourse import bacc

    f32 = mybir.dt.float32
    bf16 = mybir.dt.bfloat16
    v_dt = mybir.dt.float32r if mode == "f32r" else f32
    nc = bacc.Bacc("TRN2", target_bir_lowering=False, debug=False, num_devices=N_CORES)
    v = nc.dram_tensor("v", [SHARD, L], v_dt, kind="ExternalInput").ap()
    out = nc.dram_tensor("out", [P, 2, L], bf16, kind="ExternalOutput").ap()

    max_q = max(BLOCK_SIZES)

    with tile.TileContext(nc) as tc:
        with (
            tc.tile_pool(name="vpool", bufs=len(BLOCK_SIZES)) as vpool,
            tc.tile_pool(name="psum", bufs=2, space="PSUM") as psum_pool,
            tc.tile_pool(name="opool", bufs=1) as opool,
        ):
            g_ps = [
                psum_pool.tile([P, L], f32, tag=f"g{h}", name=f"g{h}")
                for h in range(2)
            ]

            k = 0
            r0 = 0
            for bi, blk in enumerate(BLOCK_SIZES):
                # partition p <- rows r0 + p*blk + [0, blk): blk KiB contiguous
                src = v[r0 : r0 + P * blk].rearrange("(p q) j -> p q j", q=blk)
                if mode == "bf16":
                    vt = vpool.tile([P, max_q, L], bf16, tag="v", name="vt")
                    nc.gpsimd.dma_start(vt[:, :blk, :], src)
                else:
                    vt = vpool.tile([P, max_q, L], v_dt, tag="v", name="vt")
                    # alternate between the two HWDGE rings (SP / ACT)
                    eng = nc.sync if bi % 2 == 0 else nc.scalar
                    eng.dma_start(vt[:, :blk, :], src)
                for c in range(blk):
                    rhs = vt[:, c, :]
                    for h in range(2):
                        nc.tensor.matmul(
                            g_ps[h][:],
                            vt[:, c, h * P : (h + 1) * P],
                            rhs,
                            start=(k == 0),
                            stop=(k == CHUNKS - 1),
                        )
                    k += 1
                r0 += P * blk

            o_tile = opool.tile([P, 2, L], bf16, tag="o")
            for h in range(2):
                nc.vector.tensor_copy(o_tile[:, h, :], g_ps[h][:])
            nc.sync.dma_start(out, o_tile[:])

    nc.compile()
    _CACHE[mode] = nc
    return nc


def _run(luts, W, trace=False, mode=None, **trace_kwargs):
    """Shard, run on 8 cores, return (loss_scalar, BassKernelResults)."""
    _seed_ntff_hook()
    from concourse.bass_utils import run_bass_kernel_spmd

    nc = _build(mode)

    luts = np.ascontiguousarray(np.asarray(luts, dtype=np.float32))
    W = np.asarray(W, dtype=np.float32)

    in_maps = [{"v": luts[i * SHARD : (i + 1) * SHARD]} for i in range(N_CORES)]
    res = run_bass_kernel_spmd(
        nc, in_maps, core_ids=list(range(N_CORES)), trace=trace, **trace_kwargs
    )

    # host epilogue: loss = sum(M * G_total) / NUM_LUTS  (256x256 - trivial)
    Wd = W.astype(np.float64)
    M = np.diag(Wd.sum(axis=1)) - Wd
    G = np.zeros((L, L), dtype=np.float64)
    for r in res.results:
        g = r["out"].astype(np.float64)  # [128, 2, 256]
        G[:P] += g[:, 0, :]
        G[P:] += g[:, 1, :]
    loss = np.asarray((M * G).sum() / NUM_LUTS, dtype=np.float32)
    return loss, res


def kernel(luts, W, gamma=None, **_unused):
    loss, _ = _run(luts, W, trace=False)
    return loss


if __name__ == "__main__":
    rng = np.random.default_rng(0)
    luts = rng.standard_normal((NUM_LUTS, L), dtype=np.float32)
    W = rng.random((L, L), dtype=np.float32)
    W = (W + W.T) / 2
    np.fill_diagonal(W, 0.0)
    print(kernel(luts, W))
